# revision 28
# baseline (speedup 1.0000x reference)
"""Multi-head attention (B=4, S=2048, E=1024, H=16, causal) on 8 Trainium2 cores.

Sharding: core = (batch b, head-group g) — 4 batches x 2 groups of 8 heads.
Each core computes q/k/v projections for its batch restricted to its 8 heads,
causal attention for those heads, and a partial output projection over its
512 ctx columns.  The host sums the two partials per batch and adds all
output-side bias terms analytically (softmax rows sum to one, so the v-bias
passes through attention unchanged: out += o_b + v_b @ o_w.T).

On-device layouts (per core):
  qT/kT: [head_dim 512 -> 4 tiles of 128, token 2048]  (2 heads per tile)
  v_aug: [token -> 16 tiles of 128, 8 heads x (64 dims + ones col)]
  scores are computed transposed (k^T q per head, contraction dim 64,
  two heads row-tiled concurrently in the PE array), softmax is max-free
  (scores are O(+-8), exp cannot overflow fp32), causal masking is
  structural: fully-masked tiles are skipped, diagonal tiles get an
  affine_select staircase fill.
  attn @ v is computed as v_aug^T @ expT giving ctx^T plus the softmax
  row-sum in one matmul (ones column of v_aug).

Attention is ScalarE-exp-paced (~2.2us per 2-k-tile step) while projections
are TensorE-paced; PSUM pools are kept disjoint per use (scores / proj+oproj
/ attn accumulators) so the Tile OOO scheduler can pull projection matmuls of
block n+1 into the PE idle slots of attention block n.  oproj emission is
deferred so the last (largest) attention block still has PE filler work.
"""

import os
import sys

for _p in ("/opt/trn_rl_repo", "/root/.axon_site/_ro/trn_rl_repo"):
    if os.path.isdir(_p) and _p not in sys.path:
        sys.path.append(_p)

import numpy as np
import ml_dtypes

import concourse.bacc as bacc
import concourse.mybir as mybir
from concourse import tile
from concourse import bass_utils
from concourse.bass import ts

BF16 = ml_dtypes.bfloat16
F32 = mybir.dt.float32
BF = mybir.dt.bfloat16
AFT = mybir.ActivationFunctionType
ALU = mybir.AluOpType

B, S, E = 4, 2048, 1024
H, D = 16, 64
G = 512            # head dims per core (8 heads)
KC = E // 128      # contraction chunks for projections
NM = G // 128      # m-tiles of the group dim
NJ = S // 512      # 512-wide token column blocks
NT = S // 128      # 128-wide token tiles

_NC = None


def _build():
    nc = bacc.Bacc("TRN2", target_bir_lowering=False, debug=False, num_devices=8)

    xq = nc.dram_tensor("xq", (E, S), BF, kind="ExternalInput").ap()
    xk = nc.dram_tensor("xk", (E, S), BF, kind="ExternalInput").ap()
    xv = nc.dram_tensor("xv", (E, S), BF, kind="ExternalInput").ap()
    wq = nc.dram_tensor("wq", (E, G), BF, kind="ExternalInput").ap()
    wk = nc.dram_tensor("wk", (E, G), BF, kind="ExternalInput").ap()
    wv = nc.dram_tensor("wv", (E, G), BF, kind="ExternalInput").ap()
    wo = nc.dram_tensor("wo", (G, E), BF, kind="ExternalInput").ap()
    qb = nc.dram_tensor("qb", (128, NM), F32, kind="ExternalInput").ap()
    kb = nc.dram_tensor("kb", (128, NM), F32, kind="ExternalInput").ap()
    sel = nc.dram_tensor("sel", (4, G), BF, kind="ExternalInput").ap()
    fT = nc.dram_tensor("fT", (E, S), BF, kind="ExternalOutput").ap()

    with tile.TileContext(nc) as tc:
        with (
            tc.tile_pool(name="cst", bufs=2) as cst,
            tc.tile_pool(name="wsb", bufs=24) as wsb,
            tc.tile_pool(name="xs", bufs=4) as xsp,
            tc.tile_pool(name="qt", bufs=8) as qtp,
            tc.tile_pool(name="va", bufs=16) as vap,
            tc.tile_pool(name="ctx", bufs=4) as ctxp,
            tc.tile_pool(name="exp", bufs=8) as expp,
            tc.tile_pool(name="wo", bufs=4) as wop,
            tc.tile_pool(name="fin", bufs=4) as finp,
            tc.tile_pool(name="rb", bufs=6) as rbp,
            tc.tile_pool(name="tmp", bufs=4) as tmpp,
            tc.tile_pool(name="sc", bufs=2, space="PSUM") as scp,
            tc.tile_pool(name="mm", bufs=2, space="PSUM") as mmp,
            tc.tile_pool(name="cx", bufs=2, space="PSUM") as cxp,
        ):
            qb_t = cst.tile([128, NM], F32, tag="cst")
            kb_t = cst.tile([128, NM], F32, tag="cst")
            sel_sb = cst.tile([68, G], BF, tag="sel", name="sel_sb")

            zero_fill = nc.gpsimd.to_reg(0.0)

            # Warm the ScalarE Exp table at kernel start: the table load that
            # precedes the first Exp does not reliably complete before the
            # first exp executes on a cold core, so trigger it long before
            # the real exps.
            warm = cst.tile([1, 8], F32, tag="warm", name="warm")
            nc.vector.memset(warm[:, :], 0.0)
            nc.scalar.activation(warm[:, :], warm[:, :], AFT.Exp)
            # constant ones-slots pattern for v_aug cols [64..68) per head
            ones_c = cst.tile([128, 8 * 68], BF, tag="ones", name="ones_c")
            ones_c3 = ones_c[:, :].rearrange("p (h x) -> p h x", h=8)
            nc.vector.memset(ones_c[:, :], 0.0)
            for h in range(8):
                nc.vector.memset(
                    ones_c3[:, h : h + 1, 64 + (h % 2) : 65 + (h % 2)], 1.0)

            qT = [qtp.tile([128, S], BF, tag="qt", name=f"qT{m}") for m in range(NM)]
            kT = [qtp.tile([128, S], BF, tag="qt", name=f"kT{m}") for m in range(NM)]
            ctxT = [ctxp.tile([128, S], BF, tag="ctx", name=f"ctxT{m}")
                    for m in range(NM)]
            v_aug = [None] * NT

            nc.gpsimd.dma_start(qb_t[:, :], qb[:, :])
            nc.gpsimd.dma_start(kb_t[:, :], kb[:, :])
            nc.gpsimd.dma_start(sel_sb[64:66, :], sel[0:2, :])

            # weights stay resident for the whole kernel; spread the loads
            # over two queues (gpsimd + scalar — ScalarE is idle at start) so
            # the first projection chains are not serialized behind 3.5MB on
            # one queue.
            wq_sb = [wsb.tile([128, G], BF, tag="w", name=f"wq{kc}") for kc in range(KC)]
            wk_sb = [wsb.tile([128, G], BF, tag="w", name=f"wk{kc}") for kc in range(KC)]
            wv_sb = [wsb.tile([128, G], BF, tag="w", name=f"wv{kc}") for kc in range(KC)]
            for kc in range(KC):
                nc.scalar.dma_start(wq_sb[kc][:, :], wq[ts(kc, 128), :])
            for kc in range(KC):
                nc.scalar.dma_start(wk_sb[kc][:, :], wk[ts(kc, 128), :])
            wo_sb = [wop.tile([128, E], BF, tag="wo", name=f"wo{ec}") for ec in range(NM)]

            def load_x(n, x_ap, name, eng):
                xsb = xsp.tile([128, KC * 512], BF, tag="xs", name=name)
                xs3 = xsb[:, :].rearrange("p (k c) -> p k c", k=KC)
                if n == 0:
                    # fine-grained loads so the first matmul chains start as
                    # soon as their chunk lands; xq/xk on sync, xv on gpsimd,
                    # weights on scalar — all four queues pull concurrently.
                    for kc in range(KC):
                        eng.dma_start(
                            xs3[:, kc, :], x_ap[ts(kc, 128), ts(n, 512)])
                else:
                    eng.dma_start(
                        xs3[:, :, :],
                        x_ap[:, ts(n, 512)].rearrange("(k p) c -> p k c", p=128))
                return xs3

            def proj_qk_chain(n, m, xs3, w_sb, dst, bias_t, scale):
                psd = mmp.tile([128, 512], F32, tag="mm", name="psd")
                for kc in range(KC):
                    nc.tensor.matmul(
                        psd, w_sb[kc][:, ts(m, 128)], xs3[:, kc, :],
                        start=(kc == 0), stop=(kc == KC - 1))
                nc.vector.tensor_scalar(
                    dst[m][:, ts(n, 512)], psd,
                    scale, bias_t[:, m : m + 1],
                    ALU.mult, ALU.add)

            def proj_v_chain(n, tp, xs3):
                psd = mmp.tile([128, 512], F32, tag="mm", name="psv")
                for kc in range(KC):
                    nc.tensor.matmul(
                        psd, xs3[:, kc, ts(tp, 128)], wv_sb[kc][:, :],
                        start=(kc == 0), stop=(kc == KC - 1))
                tt = 4 * n + tp
                va = vap.tile([128, 8 * 68], BF, tag="va", name=f"va{tt}")
                va3 = va[:, :].rearrange("p (h x) -> p h x", h=8)
                ps3 = psd.rearrange("p (h x) -> p h x", h=8)
                nc.vector.tensor_copy(va3[:, :, 0:64], ps3[:, :, :])
                nc.vector.tensor_copy(va3[:, :, 64:68], ones_c3[:, :, 64:68])
                v_aug[tt] = va

            def proj_block0():
                xq3 = load_x(0, xq, "xsq", nc.sync)
                xk3 = load_x(0, xk, "xsk", nc.sync)
                xv3 = load_x(0, xv, "xsv", nc.gpsimd)
                # v-weights behind the xv block-0 chunks on gpsimd; wo on
                # scalar behind wq/wk (first needed ~150us in, at oproj time)
                for kc in range(KC):
                    nc.gpsimd.dma_start(wv_sb[kc][:, :], wv[ts(kc, 128), :])
                for ec in range(NM):
                    nc.scalar.dma_start(wo_sb[ec][:, :], wo[ts(ec, 128), :])
                # HAM warm-up: the startup DMA trickle never keeps the PE
                # busy for a full 4096-cycle activity window, so without
                # these junk matmuls every projection matmul of block 0 runs
                # at the cold 1.2 GHz clock (~2x issue time).  ~7us of
                # back-to-back matmuls on resident constant data warm the
                # clock gate to 2.4 GHz while the x/w DMAs stream in.
                wps = scp.tile([128, 1024], F32, tag="sc", name="wps")
                for _ in range(18):
                    nc.tensor.matmul(
                        wps[:, 0:512], ones_c[:, 0:128], ones_c[:, 0:512],
                        start=True, stop=True)
                nc.vector.tensor_copy(warm[0:1, 0:8], wps[0:1, 0:8])
                # q/k interleaved by m-tile so attention(hp=m) unlocks after
                # chain pair m
                for m in range(NM):
                    proj_qk_chain(0, m, xq3, wq_sb, qT, qb_t, 0.125)
                    proj_qk_chain(0, m, xk3, wk_sb, kT, kb_t, 1.0)
                for tp in range(4):
                    proj_v_chain(0, tp, xv3)

            def stage_proj_fillers(n):
                # issue the x DMAs now (one trigger per tensor, three
                # different queues — each SWDGE queue moves only ~80GB/s)
                # and queue one thunk per projection chain
                xq3 = load_x(n, xq, "xsq", nc.sync)
                xk3 = load_x(n, xk, "xsk", nc.scalar)
                xv3 = load_x(n, xv, "xsv", nc.gpsimd)
                for m in range(NM):
                    filler.append(
                        lambda n=n, m=m, x=xq3: proj_qk_chain(
                            n, m, x, wq_sb, qT, qb_t, 0.125))
                    filler.append(
                        lambda n=n, m=m, x=xk3: proj_qk_chain(
                            n, m, x, wk_sb, kT, kb_t, 1.0))
                for tp in range(4):
                    filler.append(
                        lambda n=n, tp=tp, x=xv3: proj_v_chain(n, tp, x))

            # ---- manual PE-FIFO interleave ---------------------------------
            # The Tile scheduler's cost model serializes tile_position matmul
            # pairs, so in its model attention is PE-bound and it never plans
            # proj work into the (really ScalarE-exp-paced) attention phases.
            # Emission order ~= PE queue order, so we interleave explicitly:
            # `filler` holds chain-emitting thunks (proj of block n+1, oproj)
            # popped at a per-phase rate between attention ip cycles.
            filler = []
            fill_acc = [0.0]
            fill_rate = [0.0]

            def emit_fill():
                # at most one chain per cycle — a multi-chain pop blows the
                # ~2.2us exp-paced cycle budget and stalls the ScalarE stream
                fill_acc[0] = min(fill_acc[0] + fill_rate[0], 1.5)
                if fill_acc[0] >= 1.0 and filler:
                    filler.pop(0)()
                    fill_acc[0] -= 1.0

            def attention_phase(n, rs_of):
                """One flat software pipeline over all (hp, ip) cycles of
                token-column block n: scores(k) sit ahead of avs(k-1) in the
                PE FIFO (scores are ready at exp(k-1)-read-done, avs only
                after the affine staircase), fillers in the exp-latency
                shadow.  Pipelining across hp blocks avoids the ~2us stall
                per block boundary of the blocked form."""
                ni = 4 * n + 4          # causal: tk tiles 0..4j+3 (always even)
                np2 = ni // 2
                cx_of = {}

                def emit_scores(hp, ip):
                    i0, i1 = 2 * ip, 2 * ip + 1
                    sA = scp.tile([128, 1024], F32, tag="sc", name="sA")
                    sB = scp.tile([128, 1024], F32, tag="sc", name="sB")
                    for half, i in ((0, i0), (1, i1)):
                        nc.tensor.matmul(
                            sA[:, ts(half, 512)],
                            kT[hp][0:64, ts(i, 128)], qT[hp][0:64, ts(n, 512)],
                            start=True, stop=True)
                        nc.tensor.matmul(
                            sB[:, ts(half, 512)],
                            kT[hp][64:128, ts(i, 128)], qT[hp][64:128, ts(n, 512)],
                            start=True, stop=True, tile_position=(64, 0))
                    eA = expp.tile([128, 1024], BF, tag="exp", name="eA")
                    eB = expp.tile([128, 1024], BF, tag="exp", name="eB")
                    nc.scalar.activation(eA[:, :], sA[:, :], AFT.Exp)
                    nc.scalar.activation(eB[:, :], sB[:, :], AFT.Exp)
                    for half, i in ((0, i0), (1, i1)):
                        r = i - 4 * n
                        if r < 0:
                            continue
                        for e in (eA, eB):
                            # zero everything left of / above the diagonal in
                            # one pass: keep iff col - 128r - row >= 0
                            nc.gpsimd.affine_select(
                                out=e[:, 512 * half : 512 * half + 128 * (r + 1)],
                                in_=e[:, 512 * half : 512 * half + 128 * (r + 1)],
                                pattern=[[1, 128 * (r + 1)]],
                                compare_op=ALU.is_ge,
                                fill=zero_fill,
                                base=-128 * r,
                                channel_multiplier=-1)
                    return eA, eB

                def emit_avs(hp, ip, eA, eB):
                    if ip == 0:
                        cx_of[hp] = (
                            cxp.tile([68, 512], F32, tag="cx", name="cA"),
                            cxp.tile([68, 512], F32, tag="cx", name="cB"))
                    cA, cB = cx_of[hp]
                    i0, i1 = 2 * ip, 2 * ip + 1
                    for half, i in ((0, i0), (1, i1)):
                        nc.tensor.matmul(
                            cA[:, :], v_aug[i][:, (2 * hp) * 68 : (2 * hp) * 68 + 68],
                            eA[:, ts(half, 512)],
                            start=(i == 0), stop=(i == ni - 1))
                        nc.tensor.matmul(
                            cB[:, :], v_aug[i][:, (2 * hp + 1) * 68 : (2 * hp + 1) * 68 + 68],
                            eB[:, ts(half, 512)],
                            start=(i == 0), stop=(i == ni - 1))
                    if ip == np2 - 1:
                        # evacuate unnormalized ctx to SBUF; accumulate
                        # row-sums (head 2hp on row 64, 2hp+1 on row 65),
                        # then normalize this hp right away so nothing but
                        # hp=3's reciprocal is left for the phase tail
                        rs_j = rs_of[hp]
                        for c, half in ((cA, 0), (cB, 1)):
                            nc.vector.tensor_add(
                                rs_j[64:66, :], rs_j[64:66, :], c[64:66, :])
                            if half == 0:
                                nc.vector.tensor_copy(
                                    ctxT[hp][0:64, ts(n, 512)], c[0:64, :])
                            else:
                                tm = tmpp.tile([64, 512], BF, tag="tmp", name="tm")
                                nc.vector.tensor_copy(tm[:, :], c[0:64, :])
                                nc.gpsimd.dma_start(
                                    ctxT[hp][64:128, ts(n, 512)], tm[:, :])
                        normalize_hp(n, hp, rs_j)

                pend = None
                for hp in range(4):
                    for ip in range(np2):
                        e_pair = emit_scores(hp, ip)
                        emit_fill()
                        if pend is not None:
                            emit_avs(*pend)
                        pend = (hp, ip) + e_pair
                emit_avs(*pend)

            def oproj_chain(q4, e8):
                psd = mmp.tile([128, 512], F32, tag="mm", name="pso")
                for ec in range(NM):
                    nc.tensor.matmul(
                        psd, wo_sb[ec][:, ts(e8, 128)],
                        ctxT[ec][:, ts(q4, 512)],
                        start=(ec == 0), stop=(ec == NM - 1))
                st = finp.tile([128, 512], BF, tag="fin", name="st")
                nc.vector.tensor_copy(st[:, :], psd)
                nc.gpsimd.dma_start(fT[ts(e8, 128), ts(q4, 512)], st[:, :])

            def stage_oproj_fillers(q4):
                for e8 in range(8):
                    filler.append(lambda q4=q4, e8=e8: oproj_chain(q4, e8))

            def normalize_hp(j, hp, rs_x):
                # heads 2hp (row 64) and 2hp+1 (row 65) of rs_x
                rec = rbp.tile([68, 512], F32, tag="rec", name="rec", bufs=3)
                nc.vector.reciprocal(rec[64:66, :], rs_x[64:66, :])
                recb = rbp.tile([68, 512], BF, tag="recb", name="recb", bufs=3)
                nc.vector.tensor_copy(recb[64:66, :], rec[64:66, :])
                psn = mmp.tile([128, 512], F32, tag="mm", name="psn")
                nc.tensor.matmul(
                    psn,
                    sel_sb[64:66, ts(hp, 128)], recb[64:66, :],
                    start=True, stop=True, tile_position=(64, 0))
                nc.vector.tensor_mul(
                    ctxT[hp][:, ts(j, 512)], ctxT[hp][:, ts(j, 512)], psn)

            # ---- main loop ---------------------------------------------------
            # per attention phase n: fillers = projection chains of block n+1
            # plus deferred oproj chains, spread evenly over the ip cycles.
            # A few fillers are deliberately left over to keep the PE busy
            # (and the HAM clock-gate warm) through the end-of-phase
            # normalize window.
            oproj_sched = {0: (), 1: (), 2: (0,), 3: (1, 2)}
            proj_block0()
            for n in range(NJ):
                rs_of = []
                for hp in range(4):
                    r = rbp.tile([68, 512], F32, tag="rs", name=f"rs{n}_{hp}",
                                 bufs=5)
                    nc.vector.memset(r[64:66, :], 0.0)
                    rs_of.append(r)
                if n + 1 < NJ:
                    stage_proj_fillers(n + 1)
                for q4 in oproj_sched[n]:
                    stage_oproj_fillers(q4)
                cycles = 8.0 * (n + 1)
                fill_rate[0] = len(filler) / (cycles + 3.0)
                fill_acc[0] = 0.0
                attention_phase(n, rs_of)
                while filler:
                    filler.pop(0)()
            for e8 in range(8):
                oproj_chain(3, e8)

    nc.compile()
    return nc


def _get_nc():
    global _NC
    if _NC is None:
        _NC = _build()
    return _NC


def build_in_maps(inputs):
    query = np.asarray(inputs["query"], np.float32)
    key = np.asarray(inputs["key"], np.float32)
    value = np.asarray(inputs["value"], np.float32)
    q_w = np.asarray(inputs["q_w"], np.float32)
    q_b = np.asarray(inputs["q_b"], np.float32)
    k_w = np.asarray(inputs["k_w"], np.float32)
    k_b = np.asarray(inputs["k_b"], np.float32)
    v_w = np.asarray(inputs["v_w"], np.float32)
    o_w = np.asarray(inputs["o_w"], np.float32)

    xqT = [np.ascontiguousarray(query[b].T).astype(BF16) for b in range(B)]
    xkT = [np.ascontiguousarray(key[b].T).astype(BF16) for b in range(B)]
    xvT = [np.ascontiguousarray(value[b].T).astype(BF16) for b in range(B)]

    wqT, wkT, wvT, woT, qbt, kbt = [], [], [], [], [], []
    for g in range(2):
        gs = slice(g * G, (g + 1) * G)
        wqT.append(np.ascontiguousarray(q_w[gs, :].T).astype(BF16))
        wkT.append(np.ascontiguousarray(k_w[gs, :].T).astype(BF16))
        wvT.append(np.ascontiguousarray(v_w[gs, :].T).astype(BF16))
        woT.append(np.ascontiguousarray(o_w[:, gs].T).astype(BF16))
        qbt.append(
            np.ascontiguousarray((q_b[gs] / 8.0).reshape(NM, 128).T).astype(
                np.float32
            )
        )
        kbt.append(
            np.ascontiguousarray(k_b[gs].reshape(NM, 128).T).astype(np.float32)
        )

    # head 2hp -> row 0, head 2hp+1 -> row 1 (within each 128-partition band)
    sel_np = np.zeros((4, G), np.float32)
    for k in range(2):
        for p in range(G):
            if k == (p % 128) // 64:
                sel_np[k, p] = 1.0
    sel_np = sel_np.astype(BF16)

    in_maps = []
    for b in range(B):
        for g in range(2):
            in_maps.append(
                {
                    "xq": xqT[b],
                    "xk": xkT[b],
                    "xv": xvT[b],
                    "wq": wqT[g],
                    "wk": wkT[g],
                    "wv": wvT[g],
                    "wo": woT[g],
                    "qb": qbt[g],
                    "kb": kbt[g],
                    "sel": sel_np,
                }
            )

    return in_maps


def kernel(**inputs):
    nc = _get_nc()
    in_maps = build_in_maps(inputs)
    res = bass_utils.run_bass_kernel_spmd(nc, in_maps, core_ids=list(range(8)))

    o_b = np.asarray(inputs["o_b"], np.float32)
    v_b = np.asarray(inputs["v_b"], np.float32)
    o_w = np.asarray(inputs["o_w"], np.float32)
    corr = (o_b + v_b @ o_w.T).astype(np.float32)  # softmax rows sum to 1
    out = np.empty((B, S, E), np.float32)
    for b in range(B):
        acc = res.results[2 * b]["fT"].astype(np.float32) + res.results[
            2 * b + 1
        ]["fT"].astype(np.float32)
        out[b] = acc.T + corr[None, :]
    return out
